# revision 6
# baseline (speedup 1.0000x reference)
"""Trainium2 Bass kernel for nn_BatchLossFunction_38534446579748.

Loss:  cos = <pt[b,p,:], ot[b,:]> / (||pt|| * ||ot||)        (eps clamp inactive)
       v   = sigmoid(1 - cos);  gtv = gt.reshape(B,196)/255
       loss = sum(-log(1 - (v - gtv)) * (gtv*GAMMA + 1)) / B  (v > 1/2 > gtv)

Strategy (pure data parallel over batch, 8 cores x 256 batches):
  - Layout: batch on SBUF partitions, patch index on the free dim.
  - patch_tokens stream as SWDGE (gpsimd) DMAs that cast f32 -> bf16 in
    flight: HBM read bytes unchanged (memory-bound), SBUF bytes halved and
    DVE runs its packed 2 elem/cycle mode, halving per-patch dot/sumsq cost.
    Compute then trails the DMA stream instead of gating it.
  - Single activation table set for the WHOLE kernel: one explicit
    InstLoadActFuncSet(natural_log_exp_and_others) preload; square/copy/
    ln/exp all live in that set, so zero ~2.7us table switches ever happen.
      rsqrt(x)   = exp(-0.5*ln(x))        (replaces Sqrt + DVE reciprocal)
      sigmoid(y) = 1/(1+exp(-y))          (replaces Sigmoid table)
  - Per core: 2 groups of 128 batches; per group, per chunk of <=14 patches:
    ACT does most sumsq (Square, accum), DVE does dots (STT, accum) plus the
    sumsq remainder; both engines sit well under the ~15us/chunk DMA cadence.
  - g0 epilogue runs mid-stream; g1 epilogue is column-sliced so only the
    last 28 columns' short chain trails the final (tapered) DMA.
  - Partial sums [128, 3] DMA'd out; final reduction on host.
"""

import os
import sys

import numpy as np

if "/opt/trn_rl_repo" not in sys.path:
    sys.path.insert(0, "/opt/trn_rl_repo")

from contextlib import ExitStack

import concourse.bacc as bacc
import concourse.tile as tile
from concourse import mybir
from concourse.bass_utils import run_bass_kernel_spmd
from concourse.hw_specs import get_activation_tables

N_CORES = 8
B, P, D = 2048, 197, 768
NP = P - 1          # 196 usable patch tokens
BS = B // N_CORES   # 256 batches per core
PART = 128          # SBUF partitions
G = BS // PART      # 2 groups of 128 batches per core
GAMMA = 3.4

# chunk plans (patch counts); g1 tapers so the post-DMA tail is short
PLAN0 = [14] * 14
PLAN1 = [14] * 12 + [12, 8, 6, 2]
SLICE1 = 168        # g1 epilogue prefix [0:168] runs during the taper

F32 = mybir.dt.float32
BF16 = mybir.dt.bfloat16
ALU = mybir.AluOpType
ACTF = mybir.ActivationFunctionType

_CACHE = {}


def _n_act(pc):
    # squares given to ACT per chunk; the rest (plus all dots) go to DVE
    return {14: 10, 12: 9, 8: 6, 6: 4, 2: 1}[pc]


def _build():
    nc = bacc.Bacc("TRN2", target_bir_lowering=False, debug=False)

    pt = nc.dram_tensor("patch_tokens", [BS, P, D], F32, kind="ExternalInput")
    ot = nc.dram_tensor("out_text", [BS, D], F32, kind="ExternalInput")
    gt = nc.dram_tensor("gt", [BS, 14, 14], F32, kind="ExternalInput")
    out = nc.dram_tensor("loss_parts", [PART, G + 1], F32, kind="ExternalOutput")

    pt_ap = pt.ap()
    ot_ap = ot.ap()
    gt_ap = gt.ap().rearrange("b h w -> b (h w)")
    out_ap = out.ap()

    table_names = list(get_activation_tables(nc.m.arch).keys())
    nle_id = table_names.index("natural_log_exp_and_others")

    with ExitStack() as ctx:
        tc = ctx.enter_context(tile.TileContext(nc))
        xs = ctx.enter_context(tc.tile_pool(name="xs", bufs=6))
        persist = ctx.enter_context(tc.tile_pool(name="persist", bufs=1))
        psum = ctx.enter_context(tc.tile_pool(name="psum", bufs=1, space="PSUM"))

        trash_a = psum.tile([PART, D], F32, tag="trash_a")       # ACT main out
        trash_v = persist.tile([PART, D], BF16, tag="trash_v")   # DVE main out
        trash_e = persist.tile([PART, NP], F32, tag="trash_e")   # eplg main out
        loss = persist.tile([PART, G + 1], F32, tag="loss")

        # ---- preload the single activation table set used by everything ----
        nc.scalar.add_instruction(
            mybir.InstLoadActFuncSet(
                name=nc.get_next_instruction_name(),
                act_func_set_id=nle_id,
                engine=mybir.EngineType.Activation,
            )
        )

        # ---- prologue: out_text + gt on the HWDGE (sync) queue ----
        ots, otbs, otsqs, gtts, ws = [], [], [], [], []
        for g in range(G):
            b0 = g * PART
            otile = persist.tile([PART, D], F32, tag=f"ot{g}")
            nc.sync.dma_start(out=otile, in_=ot_ap[b0 : b0 + PART, :])
            ots.append(otile)
            gtt = persist.tile([PART, NP], F32, tag=f"gtt{g}")
            nc.sync.dma_start(out=gtt, in_=gt_ap[b0 : b0 + PART, :])
            gtts.append(gtt)
        for g in range(G):
            otsq = persist.tile([PART, 1], F32, tag=f"otsq{g}")
            nc.scalar.activation(
                out=trash_a, in_=ots[g], func=ACTF.Square, accum_out=otsq
            )
            otsqs.append(otsq)
            otb = persist.tile([PART, D], BF16, tag=f"otb{g}")
            nc.scalar.activation(out=otb, in_=ots[g], func=ACTF.Copy)
            otbs.append(otb)
            w = persist.tile([PART, NP], F32, tag=f"w{g}")  # -(GAMMA*gtv + 1)
            nc.scalar.activation(
                out=w, in_=gtts[g], func=ACTF.Copy, scale=-GAMMA / 255.0, bias=-1.0
            )
            ws.append(w)

        sss = [
            persist.tile([PART, NP], F32, tag=f"ss{g}", name=f"ss{g}")
            for g in range(G)
        ]
        dts = [
            persist.tile([PART, NP], F32, tag=f"dt{g}", name=f"dt{g}")
            for g in range(G)
        ]

        def chunk(g, p0, pc):
            b0 = g * PART
            x = xs.tile([PART, 14, D], BF16, tag="x")
            nc.gpsimd.dma_start(
                out=x[:, :pc, :],
                in_=pt_ap[b0 : b0 + PART, 1 + p0 : 1 + p0 + pc, :],
            )
            na = _n_act(pc)
            for j in range(pc):
                p = p0 + j
                if j < na:
                    nc.scalar.activation(
                        out=trash_a,
                        in_=x[:, j, :],
                        func=ACTF.Square,
                        accum_out=sss[g][:, p : p + 1],
                    )
                else:
                    nc.vector.scalar_tensor_tensor(
                        out=trash_v,
                        in0=x[:, j, :],
                        scalar=1.0,
                        in1=x[:, j, :],
                        op0=ALU.mult,
                        op1=ALU.mult,
                        accum_out=sss[g][:, p : p + 1],
                    )
                nc.vector.scalar_tensor_tensor(
                    out=trash_v,
                    in0=x[:, j, :],
                    scalar=1.0,
                    in1=otbs[g],
                    op0=ALU.mult,
                    op1=ALU.mult,
                    accum_out=dts[g][:, p : p + 1],
                )

        # epilogue temps (per group; slices of g1 share disjoint columns)
        t1s = [
            persist.tile([PART, NP], F32, tag=f"t1_{g}", name=f"t1_{g}")
            for g in range(G)
        ]
        t2s = [
            persist.tile([PART, NP], F32, tag=f"t2_{g}", name=f"t2_{g}")
            for g in range(G)
        ]
        t3s = [
            persist.tile([PART, NP], F32, tag=f"t3_{g}", name=f"t3_{g}")
            for g in range(G)
        ]

        def epilogue(g, lo, hi, slot):
            sl = slice(lo, hi)
            t1, t2, t3 = t1s[g], t2s[g], t3s[g]
            # den2 = ss * ||ot||^2 ; rn = rsqrt(den2) = exp(-0.5*ln(den2))
            nc.vector.tensor_scalar_mul(out=t1[:, sl], in0=sss[g][:, sl], scalar1=otsqs[g])
            nc.scalar.activation(out=t2[:, sl], in_=t1[:, sl], func=ACTF.Ln)
            nc.scalar.activation(out=t1[:, sl], in_=t2[:, sl], func=ACTF.Exp, scale=-0.5)
            # cos = dot * rn ; e = exp(cos) ; 1+exp(cos-1) = e*exp(-1) + 1
            nc.vector.tensor_mul(out=t2[:, sl], in0=dts[g][:, sl], in1=t1[:, sl])
            nc.scalar.activation(out=t3[:, sl], in_=t2[:, sl], func=ACTF.Exp)
            nc.vector.tensor_scalar(
                out=t1[:, sl],
                in0=t3[:, sl],
                scalar1=float(np.exp(-1.0)),
                scalar2=1.0,
                op0=ALU.mult,
                op1=ALU.add,
            )
            nc.vector.reciprocal(out=t2[:, sl], in_=t1[:, sl])
            # omd = gtv - v  (<0);  lnu = ln(1 + omd);  loss += lnu * w
            nc.vector.scalar_tensor_tensor(
                out=t1[:, sl],
                in0=gtts[g][:, sl],
                scalar=1.0 / 255.0,
                in1=t2[:, sl],
                op0=ALU.mult,
                op1=ALU.subtract,
            )
            nc.scalar.activation(out=t3[:, sl], in_=t1[:, sl], func=ACTF.Ln, bias=1.0)
            nc.vector.scalar_tensor_tensor(
                out=trash_e[:, sl],
                in0=t3[:, sl],
                scalar=1.0,
                in1=ws[g][:, sl],
                op0=ALU.mult,
                op1=ALU.mult,
                accum_out=loss[:, slot : slot + 1],
            )

        # ---- g0 stream + full epilogue (hidden mid-stream) ----
        p0 = 0
        for pc in PLAN0:
            chunk(0, p0, pc)
            p0 += pc
        epilogue(0, 0, NP, 0)

        # ---- g1 stream: main chunks, prefix epilogue, taper, tail ----
        p0 = 0
        for pc in PLAN1:
            if p0 == SLICE1:
                epilogue(1, 0, SLICE1, 1)
            chunk(1, p0, pc)
            p0 += pc
        epilogue(1, SLICE1, NP, 2)

        nc.sync.dma_start(out=out_ap, in_=loss)

    nc.compile()
    return nc


def _get_nc():
    if "nc" not in _CACHE:
        _CACHE["nc"] = _build()
    return _CACHE["nc"]


def _run(in_maps, **kwargs):
    return run_bass_kernel_spmd(_get_nc(), in_maps, core_ids=list(range(N_CORES)), **kwargs)


def _make_in_maps(patch_tokens, out_text, gt):
    patch_tokens = np.ascontiguousarray(np.asarray(patch_tokens, dtype=np.float32))
    out_text = np.ascontiguousarray(np.asarray(out_text, dtype=np.float32))
    gt = np.ascontiguousarray(np.asarray(gt, dtype=np.float32))
    in_maps = []
    for c in range(N_CORES):
        sl = slice(c * BS, (c + 1) * BS)
        in_maps.append(
            {
                "patch_tokens": patch_tokens[sl],
                "out_text": out_text[sl],
                "gt": gt[sl],
            }
        )
    return in_maps


def kernel(patch_tokens, out_text, gt):
    res = _run(_make_in_maps(patch_tokens, out_text, gt))
    total = np.float64(0.0)
    for r in res.results:
        total += r["loss_parts"].astype(np.float64).sum()
    return np.float32(total / B)


# revision 11
# speedup vs baseline: 1.0511x; 1.0511x over previous
"""Trainium2 Bass kernel for nn_BatchLossFunction_38534446579748.

Loss:  cos = <pt[b,p,:], ot[b,:]> / (||pt|| * ||ot||)        (eps clamp inactive)
       v   = sigmoid(1 - cos);  gtv = gt.reshape(B,196)/255
       loss = sum(-log(1 - (v - gtv)) * (gtv*GAMMA + 1)) / B  (v > 1/2 > gtv)

Strategy (pure data parallel over batch, 8 cores x 256 batches):
  - Layout: batch on SBUF partitions, patch index on the free dim.
  - patch_tokens stream as SWDGE (gpsimd) DMAs that cast f32 -> bf16 in
    flight: HBM read bytes unchanged (memory-bound), SBUF bytes halved and
    DVE runs its packed 2 elem/cycle mode, halving per-patch dot/sumsq cost.
    Compute then trails the DMA stream instead of gating it.
  - Single activation table set for the WHOLE kernel: one explicit
    InstLoadActFuncSet(natural_log_exp_and_others) preload; square/copy/
    ln/exp all live in that set, so zero ~2.7us table switches ever happen.
      rsqrt(x)   = exp(-0.5*ln(x))        (replaces Sqrt + DVE reciprocal)
      sigmoid(y) = 1/(1+exp(-y))          (replaces Sigmoid table)
  - Per core: 2 groups of 128 batches; per group, per chunk of <=14 patches:
    ACT does most sumsq (Square, accum), DVE does dots (STT, accum) plus the
    sumsq remainder; both engines sit well under the ~15us/chunk DMA cadence.
  - g0 epilogue runs mid-stream; g1 epilogue is column-sliced so only the
    last 28 columns' short chain trails the final (tapered) DMA.
  - Partial sums [128, 3] DMA'd out; final reduction on host.
"""

import os
import sys

import numpy as np

if "/opt/trn_rl_repo" not in sys.path:
    sys.path.insert(0, "/opt/trn_rl_repo")

from contextlib import ExitStack

import concourse.bacc as bacc
import concourse.tile as tile
from concourse import mybir
from concourse.bass_utils import run_bass_kernel_spmd
from concourse.hw_specs import get_activation_tables

N_CORES = 8
B, P, D = 2048, 197, 768
NP = P - 1          # 196 usable patch tokens
BS = B // N_CORES   # 256 batches per core
PART = 128          # SBUF partitions
G = BS // PART      # 2 groups of 128 batches per core
GAMMA = 3.4

# chunk plans (patch counts); the tail tapers so the post-DMA tail is short
PLAN0 = [16] * 11 + [12, 6, 2]
PLAN1 = [16] * 11 + [12, 6, 2]
SLICE1 = 188        # g1 epilogue prefix [0:188] runs during the taper

F32 = mybir.dt.float32
BF16 = mybir.dt.bfloat16
ALU = mybir.AluOpType
ACTF = mybir.ActivationFunctionType

_CACHE = {}


def _n_act(pc):
    # squares given to ACT per chunk; the rest (plus all dots) go to DVE.
    # Measured per-768-elem op: DVE dot-STT 866ns, DVE self-STT 957ns,
    # ACT square+accum-read 1119ns, cast-DMA cadence ~995ns/patch.
    return {16: 14, 12: 10, 6: 5, 2: 1}[pc]


def _build():
    nc = bacc.Bacc("TRN2", target_bir_lowering=False, debug=False)

    pt = nc.dram_tensor("patch_tokens", [BS, P, D], F32, kind="ExternalInput")
    ot = nc.dram_tensor("out_text", [BS, D], F32, kind="ExternalInput")
    gt = nc.dram_tensor("gt", [BS, 14, 14], F32, kind="ExternalInput")
    out = nc.dram_tensor("loss_parts", [PART, G + 1], F32, kind="ExternalOutput")

    pt_ap = pt.ap()
    ot_ap = ot.ap()
    gt_ap = gt.ap().rearrange("b h w -> b (h w)")
    out_ap = out.ap()

    table_names = list(get_activation_tables(nc.m.arch).keys())
    nle_id = table_names.index("natural_log_exp_and_others")

    with ExitStack() as ctx:
        tc = ctx.enter_context(tile.TileContext(nc))
        xs = ctx.enter_context(tc.tile_pool(name="xs", bufs=7))
        persist = ctx.enter_context(tc.tile_pool(name="persist", bufs=1))
        psum = ctx.enter_context(tc.tile_pool(name="psum", bufs=1, space="PSUM"))

        trash_a = psum.tile([PART, D], F32, tag="trash_a")       # ACT main out
        trash_v = persist.tile([PART, D], BF16, tag="trash_v")   # DVE main out
        trash_e = persist.tile([PART, NP], F32, tag="trash_e")   # eplg main out
        loss = persist.tile([PART, G + 1], F32, tag="loss")

        # ---- preload the single activation table set used by everything ----
        nc.scalar.add_instruction(
            mybir.InstLoadActFuncSet(
                name=nc.get_next_instruction_name(),
                act_func_set_id=nle_id,
                engine=mybir.EngineType.Activation,
            )
        )

        # ---- prologue: out_text + gt on the HWDGE (sync) queue ----
        ots, otbs, otsqs, gtts, ws = [], [], [], [], []
        for g in range(G):
            b0 = g * PART
            otile = persist.tile([PART, D], F32, tag=f"ot{g}")
            nc.sync.dma_start(out=otile, in_=ot_ap[b0 : b0 + PART, :])
            ots.append(otile)
            gtt = persist.tile([PART, NP], F32, tag=f"gtt{g}")
            nc.sync.dma_start(out=gtt, in_=gt_ap[b0 : b0 + PART, :])
            gtts.append(gtt)
        for g in range(G):
            otsq = persist.tile([PART, 1], F32, tag=f"otsq{g}")
            nc.scalar.activation(
                out=trash_a, in_=ots[g], func=ACTF.Square, accum_out=otsq
            )
            otsqs.append(otsq)
            otb = persist.tile([PART, D], BF16, tag=f"otb{g}")
            nc.scalar.activation(out=otb, in_=ots[g], func=ACTF.Copy)
            otbs.append(otb)
            w = persist.tile([PART, NP], F32, tag=f"w{g}")  # -(GAMMA*gtv + 1)
            nc.scalar.activation(
                out=w, in_=gtts[g], func=ACTF.Copy, scale=-GAMMA / 255.0, bias=-1.0
            )
            ws.append(w)

        sss = [
            persist.tile([PART, NP], F32, tag=f"ss{g}", name=f"ss{g}")
            for g in range(G)
        ]
        dts = [
            persist.tile([PART, NP], F32, tag=f"dt{g}", name=f"dt{g}")
            for g in range(G)
        ]

        def chunk(g, p0, pc):
            b0 = g * PART
            x = xs.tile([PART, 16, D], BF16, tag="x")
            nc.gpsimd.dma_start(
                out=x[:, :pc, :],
                in_=pt_ap[b0 : b0 + PART, 1 + p0 : 1 + p0 + pc, :],
            )
            na = _n_act(pc)
            for j in range(pc):
                p = p0 + j
                if j < na:
                    nc.scalar.activation(
                        out=trash_a,
                        in_=x[:, j, :],
                        func=ACTF.Square,
                        accum_out=sss[g][:, p : p + 1],
                    )
                else:
                    nc.vector.scalar_tensor_tensor(
                        out=trash_v,
                        in0=x[:, j, :],
                        scalar=1.0,
                        in1=x[:, j, :],
                        op0=ALU.mult,
                        op1=ALU.mult,
                        accum_out=sss[g][:, p : p + 1],
                    )
                nc.vector.scalar_tensor_tensor(
                    out=trash_v,
                    in0=x[:, j, :],
                    scalar=1.0,
                    in1=otbs[g],
                    op0=ALU.mult,
                    op1=ALU.mult,
                    accum_out=dts[g][:, p : p + 1],
                )

        # epilogue temps (per group; slices of g1 share disjoint columns)
        t1s = [
            persist.tile([PART, NP], F32, tag=f"t1_{g}", name=f"t1_{g}")
            for g in range(G)
        ]
        t2s = [
            persist.tile([PART, NP], F32, tag=f"t2_{g}", name=f"t2_{g}")
            for g in range(G)
        ]
        t3s = [
            persist.tile([PART, NP], F32, tag=f"t3_{g}", name=f"t3_{g}")
            for g in range(G)
        ]

        def epilogue(g, lo, hi, slot):
            sl = slice(lo, hi)
            t1, t2, t3 = t1s[g], t2s[g], t3s[g]
            # den2 = ss * ||ot||^2 ; rn = rsqrt(den2) = exp(-0.5*ln(den2))
            nc.vector.tensor_scalar_mul(out=t1[:, sl], in0=sss[g][:, sl], scalar1=otsqs[g])
            nc.scalar.activation(out=t2[:, sl], in_=t1[:, sl], func=ACTF.Ln)
            nc.scalar.activation(out=t1[:, sl], in_=t2[:, sl], func=ACTF.Exp, scale=-0.5)
            # cos = dot * rn ; e = exp(cos) ; 1+exp(cos-1) = e*exp(-1) + 1
            nc.vector.tensor_mul(out=t2[:, sl], in0=dts[g][:, sl], in1=t1[:, sl])
            nc.scalar.activation(out=t3[:, sl], in_=t2[:, sl], func=ACTF.Exp)
            nc.vector.tensor_scalar(
                out=t1[:, sl],
                in0=t3[:, sl],
                scalar1=float(np.exp(-1.0)),
                scalar2=1.0,
                op0=ALU.mult,
                op1=ALU.add,
            )
            nc.vector.reciprocal_approx_fast(out=t2[:, sl], in_=t1[:, sl])
            # omd = gtv - v  (<0);  lnu = ln(1 + omd);  loss += lnu * w
            nc.vector.scalar_tensor_tensor(
                out=t1[:, sl],
                in0=gtts[g][:, sl],
                scalar=1.0 / 255.0,
                in1=t2[:, sl],
                op0=ALU.mult,
                op1=ALU.subtract,
            )
            nc.scalar.activation(out=t3[:, sl], in_=t1[:, sl], func=ACTF.Ln, bias=1.0)
            nc.vector.scalar_tensor_tensor(
                out=trash_e[:, sl],
                in0=t3[:, sl],
                scalar=1.0,
                in1=ws[g][:, sl],
                op0=ALU.mult,
                op1=ALU.mult,
                accum_out=loss[:, slot : slot + 1],
            )

        # ---- g0 stream + full epilogue (hidden mid-stream) ----
        p0 = 0
        for pc in PLAN0:
            chunk(0, p0, pc)
            p0 += pc
        epilogue(0, 0, NP, 0)

        # ---- g1 stream: main chunks, prefix epilogue, taper, tail ----
        p0 = 0
        for pc in PLAN1:
            if p0 == SLICE1:
                epilogue(1, 0, SLICE1, 1)
            chunk(1, p0, pc)
            p0 += pc
        epilogue(1, SLICE1, NP, 2)

        nc.sync.dma_start(out=out_ap, in_=loss)

    nc.compile()
    return nc


def _get_nc():
    if "nc" not in _CACHE:
        _CACHE["nc"] = _build()
    return _CACHE["nc"]


def _run(in_maps, **kwargs):
    return run_bass_kernel_spmd(_get_nc(), in_maps, core_ids=list(range(N_CORES)), **kwargs)


def _make_in_maps(patch_tokens, out_text, gt):
    patch_tokens = np.ascontiguousarray(np.asarray(patch_tokens, dtype=np.float32))
    out_text = np.ascontiguousarray(np.asarray(out_text, dtype=np.float32))
    gt = np.ascontiguousarray(np.asarray(gt, dtype=np.float32))
    in_maps = []
    for c in range(N_CORES):
        sl = slice(c * BS, (c + 1) * BS)
        in_maps.append(
            {
                "patch_tokens": patch_tokens[sl],
                "out_text": out_text[sl],
                "gt": gt[sl],
            }
        )
    return in_maps


def kernel(patch_tokens, out_text, gt):
    res = _run(_make_in_maps(patch_tokens, out_text, gt))
    total = np.float64(0.0)
    for r in res.results:
        total += r["loss_parts"].astype(np.float64).sum()
    return np.float32(total / B)


# revision 15
# speedup vs baseline: 1.0954x; 1.0422x over previous
"""Trainium2 Bass kernel for nn_BatchLossFunction_38534446579748.

Loss:  cos = <pt[b,p,:], ot[b,:]> / (||pt|| * ||ot||)        (eps clamp inactive)
       v   = sigmoid(1 - cos);  gtv = gt.reshape(B,196)/255
       loss = sum(-log(1 - (v - gtv)) * (gtv*GAMMA + 1)) / B  (v > 1/2 > gtv)

Strategy (pure data parallel over batch, 8 cores x 256 batches):
  - Layout: batch on SBUF partitions, patch index on the free dim.
  - patch_tokens stream as SWDGE (gpsimd) DMAs that cast f32 -> bf16 in
    flight: HBM read bytes unchanged (memory-bound), SBUF bytes halved and
    DVE runs its packed 2 elem/cycle mode, halving per-patch dot/sumsq cost.
    Compute then trails the DMA stream instead of gating it.
  - Single activation table set for the WHOLE kernel: one explicit
    InstLoadActFuncSet(natural_log_exp_and_others) preload; square/copy/
    ln/exp all live in that set, so zero ~2.7us table switches ever happen.
      rsqrt(x)   = exp(-0.5*ln(x))        (replaces Sqrt + DVE reciprocal)
      sigmoid(y) = 1/(1+exp(-y))          (replaces Sigmoid table)
  - Per core: 2 groups of 128 batches; per group, per chunk of <=14 patches:
    ACT does most sumsq (Square, accum), DVE does dots (STT, accum) plus the
    sumsq remainder; both engines sit well under the ~15us/chunk DMA cadence.
  - g0 epilogue runs mid-stream; g1 epilogue is column-sliced so only the
    last 28 columns' short chain trails the final (tapered) DMA.
  - Partial sums [128, 3] DMA'd out; final reduction on host.
"""

import os
import sys

import numpy as np

if "/opt/trn_rl_repo" not in sys.path:
    sys.path.insert(0, "/opt/trn_rl_repo")

from contextlib import ExitStack

import concourse.bacc as bacc
import concourse.tile as tile
from concourse import mybir
from concourse.bass_utils import run_bass_kernel_spmd
from concourse.hw_specs import get_activation_tables

N_CORES = 8
B, P, D = 2048, 197, 768
NP = P - 1          # 196 usable patch tokens
BS = B // N_CORES   # 256 batches per core
PART = 128          # SBUF partitions
G = BS // PART      # 2 groups of 128 batches per core
GAMMA = 3.4

# chunk plans (patch counts); the tail tapers so the post-DMA tail is short.
# 14-patch chunks keep the 16 SDMA engines load-uniform (16-patch chunks
# provoked a 22% skew on engine 15).
PLAN0 = [14] * 13 + [8, 4, 2]
PLAN1 = [14] * 13 + [8, 4, 2]
SLICE1 = 182        # g1 epilogue prefix [0:182] runs during the taper

F32 = mybir.dt.float32
BF16 = mybir.dt.bfloat16
ALU = mybir.AluOpType
ACTF = mybir.ActivationFunctionType

_CACHE = {}


def _n_act(pc):
    # squares given to ACT per chunk; the rest (plus all dots) go to DVE.
    # Measured per-768-elem op: DVE STT 957ns (accum forces 1x mode; bf16
    # gives no speedup), ACT square+accum-read 1119ns. Balancing the two
    # engines puts ~34 of 392 sumsq on DVE: total ~411us each, just above
    # the ~393us DMA stream, so compute is the (thin) critical path.
    return {14: 13, 8: 6, 4: 3, 2: 1}[pc]


def _build():
    nc = bacc.Bacc("TRN2", target_bir_lowering=False, debug=False)

    pt = nc.dram_tensor("patch_tokens", [BS, P, D], F32, kind="ExternalInput")
    ot = nc.dram_tensor("out_text", [BS, D], F32, kind="ExternalInput")
    gt = nc.dram_tensor("gt", [BS, 14, 14], F32, kind="ExternalInput")
    out = nc.dram_tensor("loss_parts", [PART, G + 1], F32, kind="ExternalOutput")

    pt_ap = pt.ap()
    ot_ap = ot.ap()
    gt_ap = gt.ap().rearrange("b h w -> b (h w)")
    out_ap = out.ap()

    table_names = list(get_activation_tables(nc.m.arch).keys())
    nle_id = table_names.index("natural_log_exp_and_others")

    with ExitStack() as ctx:
        tc = ctx.enter_context(tile.TileContext(nc))
        xs = ctx.enter_context(tc.tile_pool(name="xs", bufs=8))
        persist = ctx.enter_context(tc.tile_pool(name="persist", bufs=1))
        psum = ctx.enter_context(tc.tile_pool(name="psum", bufs=1, space="PSUM"))

        trash_a = psum.tile([PART, D], F32, tag="trash_a")       # ACT main out
        trash_v = persist.tile([PART, D], BF16, tag="trash_v")   # DVE main out
        trash_e = persist.tile([PART, NP], F32, tag="trash_e")   # eplg main out
        loss = persist.tile([PART, G + 1], F32, tag="loss")

        # ---- preload the single activation table set used by everything ----
        nc.scalar.add_instruction(
            mybir.InstLoadActFuncSet(
                name=nc.get_next_instruction_name(),
                act_func_set_id=nle_id,
                engine=mybir.EngineType.Activation,
            )
        )

        # ---- prologue: out_text + gt on the HWDGE (sync) queue ----
        ots, otbs, otsqs, gtts, ws = [], [], [], [], []
        for g in range(G):
            b0 = g * PART
            otile = persist.tile([PART, D], F32, tag=f"ot{g}")
            nc.sync.dma_start(out=otile, in_=ot_ap[b0 : b0 + PART, :])
            ots.append(otile)
            gtt = persist.tile([PART, NP], F32, tag=f"gtt{g}")
            nc.sync.dma_start(out=gtt, in_=gt_ap[b0 : b0 + PART, :])
            gtts.append(gtt)
        for g in range(G):
            otsq = persist.tile([PART, 1], F32, tag=f"otsq{g}")
            nc.scalar.activation(
                out=trash_a, in_=ots[g], func=ACTF.Square, accum_out=otsq
            )
            otsqs.append(otsq)
            otb = persist.tile([PART, D], BF16, tag=f"otb{g}")
            nc.scalar.activation(out=otb, in_=ots[g], func=ACTF.Copy)
            otbs.append(otb)
            w = persist.tile([PART, NP], F32, tag=f"w{g}")  # -(GAMMA*gtv + 1)
            nc.scalar.activation(
                out=w, in_=gtts[g], func=ACTF.Copy, scale=-GAMMA / 255.0, bias=-1.0
            )
            ws.append(w)

        sss = [
            persist.tile([PART, NP], F32, tag=f"ss{g}", name=f"ss{g}")
            for g in range(G)
        ]
        dts = [
            persist.tile([PART, NP], F32, tag=f"dt{g}", name=f"dt{g}")
            for g in range(G)
        ]

        def chunk(g, p0, pc):
            b0 = g * PART
            x = xs.tile([PART, 14, D], BF16, tag="x")
            nc.gpsimd.dma_start(
                out=x[:, :pc, :],
                in_=pt_ap[b0 : b0 + PART, 1 + p0 : 1 + p0 + pc, :],
            )
            na = _n_act(pc)
            for j in range(pc):
                p = p0 + j
                if j < na:
                    nc.scalar.activation(
                        out=trash_a,
                        in_=x[:, j, :],
                        func=ACTF.Square,
                        accum_out=sss[g][:, p : p + 1],
                    )
                else:
                    nc.vector.scalar_tensor_tensor(
                        out=trash_v,
                        in0=x[:, j, :],
                        scalar=1.0,
                        in1=x[:, j, :],
                        op0=ALU.mult,
                        op1=ALU.mult,
                        accum_out=sss[g][:, p : p + 1],
                    )
                nc.vector.scalar_tensor_tensor(
                    out=trash_v,
                    in0=x[:, j, :],
                    scalar=1.0,
                    in1=otbs[g],
                    op0=ALU.mult,
                    op1=ALU.mult,
                    accum_out=dts[g][:, p : p + 1],
                )

        # epilogue temps (per group; slices of g1 share disjoint columns)
        t1s = [
            persist.tile([PART, NP], F32, tag=f"t1_{g}", name=f"t1_{g}")
            for g in range(G)
        ]
        t2s = [
            persist.tile([PART, NP], F32, tag=f"t2_{g}", name=f"t2_{g}")
            for g in range(G)
        ]
        t3s = [
            persist.tile([PART, NP], F32, tag=f"t3_{g}", name=f"t3_{g}")
            for g in range(G)
        ]

        def epilogue(g, lo, hi, slot):
            sl = slice(lo, hi)
            t1, t2, t3 = t1s[g], t2s[g], t3s[g]
            # den2 = ss * ||ot||^2 ; rn = rsqrt(den2) = exp(-0.5*ln(den2))
            nc.vector.tensor_scalar_mul(out=t1[:, sl], in0=sss[g][:, sl], scalar1=otsqs[g])
            nc.scalar.activation(out=t2[:, sl], in_=t1[:, sl], func=ACTF.Ln)
            nc.scalar.activation(out=t1[:, sl], in_=t2[:, sl], func=ACTF.Exp, scale=-0.5)
            # cos = dot * rn ; e = exp(cos) ; 1+exp(cos-1) = e*exp(-1) + 1
            nc.vector.tensor_mul(out=t2[:, sl], in0=dts[g][:, sl], in1=t1[:, sl])
            nc.scalar.activation(out=t3[:, sl], in_=t2[:, sl], func=ACTF.Exp)
            nc.vector.tensor_scalar(
                out=t1[:, sl],
                in0=t3[:, sl],
                scalar1=float(np.exp(-1.0)),
                scalar2=1.0,
                op0=ALU.mult,
                op1=ALU.add,
            )
            nc.vector.reciprocal_approx_fast(out=t2[:, sl], in_=t1[:, sl])
            # omd = gtv - v  (<0);  lnu = ln(1 + omd);  loss += lnu * w
            nc.vector.scalar_tensor_tensor(
                out=t1[:, sl],
                in0=gtts[g][:, sl],
                scalar=1.0 / 255.0,
                in1=t2[:, sl],
                op0=ALU.mult,
                op1=ALU.subtract,
            )
            nc.scalar.activation(out=t3[:, sl], in_=t1[:, sl], func=ACTF.Ln, bias=1.0)
            nc.vector.scalar_tensor_tensor(
                out=trash_e[:, sl],
                in0=t3[:, sl],
                scalar=1.0,
                in1=ws[g][:, sl],
                op0=ALU.mult,
                op1=ALU.mult,
                accum_out=loss[:, slot : slot + 1],
            )

        # ---- g0 stream + full epilogue (hidden mid-stream) ----
        p0 = 0
        for pc in PLAN0:
            chunk(0, p0, pc)
            p0 += pc
        epilogue(0, 0, NP, 0)

        # ---- g1 stream: main chunks, prefix epilogue, taper, tail ----
        p0 = 0
        for pc in PLAN1:
            if p0 == SLICE1:
                epilogue(1, 0, SLICE1, 1)
            chunk(1, p0, pc)
            p0 += pc
        epilogue(1, SLICE1, NP, 2)

        nc.sync.dma_start(out=out_ap, in_=loss)

    nc.compile()
    return nc


def _get_nc():
    if "nc" not in _CACHE:
        _CACHE["nc"] = _build()
    return _CACHE["nc"]


def _run(in_maps, **kwargs):
    return run_bass_kernel_spmd(_get_nc(), in_maps, core_ids=list(range(N_CORES)), **kwargs)


def _make_in_maps(patch_tokens, out_text, gt):
    patch_tokens = np.ascontiguousarray(np.asarray(patch_tokens, dtype=np.float32))
    out_text = np.ascontiguousarray(np.asarray(out_text, dtype=np.float32))
    gt = np.ascontiguousarray(np.asarray(gt, dtype=np.float32))
    in_maps = []
    for c in range(N_CORES):
        sl = slice(c * BS, (c + 1) * BS)
        in_maps.append(
            {
                "patch_tokens": patch_tokens[sl],
                "out_text": out_text[sl],
                "gt": gt[sl],
            }
        )
    return in_maps


def kernel(patch_tokens, out_text, gt):
    res = _run(_make_in_maps(patch_tokens, out_text, gt))
    total = np.float64(0.0)
    for r in res.results:
        total += r["loss_parts"].astype(np.float64).sum()
    return np.float32(total / B)


# revision 22
# speedup vs baseline: 1.1357x; 1.0368x over previous
"""Trainium2 Bass kernel for nn_BatchLossFunction_38534446579748.

Loss:  cos = <pt[b,p,:], ot[b,:]> / (||pt|| * ||ot||)        (eps clamp inactive)
       v   = sigmoid(1 - cos);  gtv = gt.reshape(B,196)/255
       loss = sum(-log(1 - (v - gtv)) * (gtv*GAMMA + 1)) / B  (v > 1/2 > gtv)

Strategy (pure data parallel over batch, 8 cores x 256 batches):
  - Layout: batch on SBUF partitions, patch index on the free dim.
  - patch_tokens stream as f32 HWDGE (sync) DMAs, 14-patch chunks
    (~5.5 MB each). HWDGE avoids the SWDGE descriptor-ring port contention
    that slows SDMA engine 15 by ~20% and avoids any benefitless cast
    (DVE accum ops run at 1 elem/cycle regardless of dtype).
  - Single activation table set for the WHOLE kernel: one explicit
    InstLoadActFuncSet(natural_log_exp_and_others) preload; square/copy/
    ln/exp all live in that set, so zero ~2.7us table switches ever happen.
      rsqrt(x)   = exp(-0.5*ln(x))        (replaces Sqrt + DVE reciprocal)
      sigmoid(y) = 1/(1+exp(-y))          (replaces Sigmoid table)
  - Engine balance (measured: DVE STT-accum 957ns, ACT square+accum-read
    1119ns per 768-elem op): DVE takes all dots plus ~1 sumsq per chunk,
    ACT takes 13 sumsq per 14-patch chunk -> both engines total ~411us,
    just under the ~415us HBM stream.
  - g0 epilogue runs mid-stream; g1 epilogue is column-sliced so only the
    last 14 columns' short chain (~2.5us, no table loads) trails the final
    (tapered) DMA.
  - Partial sums [128, 3] DMA'd out; final reduction on host.
"""

import os
import sys

import numpy as np

if "/opt/trn_rl_repo" not in sys.path:
    sys.path.insert(0, "/opt/trn_rl_repo")

from contextlib import ExitStack

import concourse.bacc as bacc
import concourse.tile as tile
from concourse import mybir
from concourse.bass_utils import run_bass_kernel_spmd
from concourse.hw_specs import get_activation_tables

N_CORES = 8
B, P, D = 2048, 197, 768
NP = P - 1          # 196 usable patch tokens
BS = B // N_CORES   # 256 batches per core
PART = 128          # SBUF partitions
G = BS // PART      # 2 groups of 128 batches per core
GAMMA = 3.4

# chunk plans (patch counts); the tail tapers so the post-DMA tail is short.
# 14-patch chunks keep the 16 SDMA engines load-uniform (16-patch chunks
# provoked a 22% skew on engine 15).
PLAN0 = [14] * 13 + [8, 4, 2]
PLAN1 = [14] * 13 + [8, 4, 2]
SLICE1 = 182        # g1 epilogue prefix [0:182] runs during the taper

F32 = mybir.dt.float32
BF16 = mybir.dt.bfloat16
ALU = mybir.AluOpType
ACTF = mybir.ActivationFunctionType

_CACHE = {}


def _n_act(pc):
    # squares given to ACT per chunk; the rest (plus all dots) go to DVE.
    # Measured per-768-elem op: DVE STT 957ns (accum forces 1x mode; bf16
    # gives no speedup), ACT square+accum-read 1119ns. Balancing the two
    # engines puts ~34 of 392 sumsq on DVE: total ~411us each, just above
    # the ~393us DMA stream, so compute is the (thin) critical path.
    return {14: 13, 8: 6, 4: 3, 2: 1}[pc]


def _build():
    nc = bacc.Bacc("TRN2", target_bir_lowering=False, debug=False)

    pt = nc.dram_tensor("patch_tokens", [BS, P, D], F32, kind="ExternalInput")
    ot = nc.dram_tensor("out_text", [BS, D], F32, kind="ExternalInput")
    gt = nc.dram_tensor("gt", [BS, 14, 14], F32, kind="ExternalInput")
    out = nc.dram_tensor("loss_parts", [PART, G + 1], F32, kind="ExternalOutput")

    pt_ap = pt.ap()
    ot_ap = ot.ap()
    gt_ap = gt.ap().rearrange("b h w -> b (h w)")
    out_ap = out.ap()

    table_names = list(get_activation_tables(nc.m.arch).keys())
    nle_id = table_names.index("natural_log_exp_and_others")

    with ExitStack() as ctx:
        tc = ctx.enter_context(tile.TileContext(nc))
        xs = ctx.enter_context(tc.tile_pool(name="xs", bufs=4))
        persist = ctx.enter_context(tc.tile_pool(name="persist", bufs=1))
        psum = ctx.enter_context(tc.tile_pool(name="psum", bufs=1, space="PSUM"))

        trash_a = psum.tile([PART, D], F32, tag="trash_a")       # ACT main out
        trash_v = persist.tile([PART, D], F32, tag="trash_v")    # DVE main out
        trash_e = persist.tile([PART, NP], F32, tag="trash_e")   # eplg main out
        loss = persist.tile([PART, G + 1], F32, tag="loss")

        # ---- preload the single activation table set used by everything ----
        nc.scalar.add_instruction(
            mybir.InstLoadActFuncSet(
                name=nc.get_next_instruction_name(),
                act_func_set_id=nle_id,
                engine=mybir.EngineType.Activation,
            )
        )

        # ---- prologue: out_text + gt on the HWDGE (sync) queue ----
        ots, otsqs, gtts, ws = [], [], [], []
        for g in range(G):
            b0 = g * PART
            otile = persist.tile([PART, D], F32, tag=f"ot{g}")
            nc.sync.dma_start(out=otile, in_=ot_ap[b0 : b0 + PART, :])
            ots.append(otile)
            gtt = persist.tile([PART, NP], F32, tag=f"gtt{g}")
            nc.sync.dma_start(out=gtt, in_=gt_ap[b0 : b0 + PART, :])
            gtts.append(gtt)
        for g in range(G):
            otsq = persist.tile([PART, 1], F32, tag=f"otsq{g}")
            nc.scalar.activation(
                out=trash_a, in_=ots[g], func=ACTF.Square, accum_out=otsq
            )
            otsqs.append(otsq)
            w = persist.tile([PART, NP], F32, tag=f"w{g}")  # -(GAMMA*gtv + 1)
            nc.scalar.activation(
                out=w, in_=gtts[g], func=ACTF.Copy, scale=-GAMMA / 255.0, bias=-1.0
            )
            ws.append(w)

        sss = [
            persist.tile([PART, NP], F32, tag=f"ss{g}", name=f"ss{g}")
            for g in range(G)
        ]
        dts = [
            persist.tile([PART, NP], F32, tag=f"dt{g}", name=f"dt{g}")
            for g in range(G)
        ]

        def chunk(g, p0, pc):
            b0 = g * PART
            x = xs.tile([PART, 14, D], F32, tag="x")
            nc.sync.dma_start(
                out=x[:, :pc, :],
                in_=pt_ap[b0 : b0 + PART, 1 + p0 : 1 + p0 + pc, :],
            )
            na = _n_act(pc)
            for j in range(pc):
                p = p0 + j
                if j < na:
                    nc.scalar.activation(
                        out=trash_a,
                        in_=x[:, j, :],
                        func=ACTF.Square,
                        accum_out=sss[g][:, p : p + 1],
                    )
                else:
                    nc.vector.scalar_tensor_tensor(
                        out=trash_v,
                        in0=x[:, j, :],
                        scalar=1.0,
                        in1=x[:, j, :],
                        op0=ALU.mult,
                        op1=ALU.mult,
                        accum_out=sss[g][:, p : p + 1],
                    )
                nc.vector.scalar_tensor_tensor(
                    out=trash_v,
                    in0=x[:, j, :],
                    scalar=1.0,
                    in1=ots[g],
                    op0=ALU.mult,
                    op1=ALU.mult,
                    accum_out=dts[g][:, p : p + 1],
                )

        # epilogue temps (per group; slices of g1 share disjoint columns)
        t1s = [
            persist.tile([PART, NP], F32, tag=f"t1_{g}", name=f"t1_{g}")
            for g in range(G)
        ]
        t2s = [
            persist.tile([PART, NP], F32, tag=f"t2_{g}", name=f"t2_{g}")
            for g in range(G)
        ]
        t3s = [
            persist.tile([PART, NP], F32, tag=f"t3_{g}", name=f"t3_{g}")
            for g in range(G)
        ]

        def epilogue(g, lo, hi, slot):
            sl = slice(lo, hi)
            t1, t2, t3 = t1s[g], t2s[g], t3s[g]
            # den2 = ss * ||ot||^2 ; rn = rsqrt(den2) = exp(-0.5*ln(den2))
            nc.vector.tensor_scalar_mul(out=t1[:, sl], in0=sss[g][:, sl], scalar1=otsqs[g])
            nc.scalar.activation(out=t2[:, sl], in_=t1[:, sl], func=ACTF.Ln)
            nc.scalar.activation(out=t1[:, sl], in_=t2[:, sl], func=ACTF.Exp, scale=-0.5)
            # cos = dot * rn ; e = exp(cos) ; 1+exp(cos-1) = e*exp(-1) + 1
            nc.vector.tensor_mul(out=t2[:, sl], in0=dts[g][:, sl], in1=t1[:, sl])
            nc.scalar.activation(out=t3[:, sl], in_=t2[:, sl], func=ACTF.Exp)
            nc.vector.tensor_scalar(
                out=t1[:, sl],
                in0=t3[:, sl],
                scalar1=float(np.exp(-1.0)),
                scalar2=1.0,
                op0=ALU.mult,
                op1=ALU.add,
            )
            nc.vector.reciprocal_approx_fast(out=t2[:, sl], in_=t1[:, sl])
            # omd = gtv - v  (<0);  lnu = ln(1 + omd);  loss += lnu * w
            nc.vector.scalar_tensor_tensor(
                out=t1[:, sl],
                in0=gtts[g][:, sl],
                scalar=1.0 / 255.0,
                in1=t2[:, sl],
                op0=ALU.mult,
                op1=ALU.subtract,
            )
            nc.scalar.activation(out=t3[:, sl], in_=t1[:, sl], func=ACTF.Ln, bias=1.0)
            nc.vector.scalar_tensor_tensor(
                out=trash_e[:, sl],
                in0=t3[:, sl],
                scalar=1.0,
                in1=ws[g][:, sl],
                op0=ALU.mult,
                op1=ALU.mult,
                accum_out=loss[:, slot : slot + 1],
            )

        # ---- g0 stream + full epilogue (hidden mid-stream) ----
        p0 = 0
        for pc in PLAN0:
            chunk(0, p0, pc)
            p0 += pc
        epilogue(0, 0, NP, 0)

        # ---- g1 stream: main chunks, prefix epilogue, taper, tail ----
        p0 = 0
        for pc in PLAN1:
            if p0 == SLICE1:
                epilogue(1, 0, SLICE1, 1)
            chunk(1, p0, pc)
            p0 += pc
        epilogue(1, SLICE1, NP, 2)

        nc.sync.dma_start(out=out_ap, in_=loss)

    nc.compile()
    return nc


def _get_nc():
    if "nc" not in _CACHE:
        _CACHE["nc"] = _build()
    return _CACHE["nc"]


def _run(in_maps, **kwargs):
    return run_bass_kernel_spmd(_get_nc(), in_maps, core_ids=list(range(N_CORES)), **kwargs)


def _make_in_maps(patch_tokens, out_text, gt):
    patch_tokens = np.ascontiguousarray(np.asarray(patch_tokens, dtype=np.float32))
    out_text = np.ascontiguousarray(np.asarray(out_text, dtype=np.float32))
    gt = np.ascontiguousarray(np.asarray(gt, dtype=np.float32))
    in_maps = []
    for c in range(N_CORES):
        sl = slice(c * BS, (c + 1) * BS)
        in_maps.append(
            {
                "patch_tokens": patch_tokens[sl],
                "out_text": out_text[sl],
                "gt": gt[sl],
            }
        )
    return in_maps


def kernel(patch_tokens, out_text, gt):
    res = _run(_make_in_maps(patch_tokens, out_text, gt))
    total = np.float64(0.0)
    for r in res.results:
        total += r["loss_parts"].astype(np.float64).sum()
    return np.float32(total / B)


# revision 25
# speedup vs baseline: 1.1835x; 1.0421x over previous
"""Trainium2 Bass kernel for nn_BatchLossFunction_38534446579748.

Loss:  cos = <pt[b,p,:], ot[b,:]> / (||pt|| * ||ot||)        (eps clamp inactive)
       v   = sigmoid(1 - cos);  gtv = gt.reshape(B,196)/255
       loss = sum(-log(1 - (v - gtv)) * (gtv*GAMMA + 1)) / B  (v > 1/2 > gtv)

Strategy (pure data parallel over batch, 8 cores x 256 batches):
  - Layout: batch on SBUF partitions, patch index on the free dim.
  - patch_tokens stream as f32 HWDGE (sync) DMAs, 14-patch chunks
    (~5.5 MB each). HWDGE avoids the SWDGE descriptor-ring port contention
    that slows SDMA engine 15 by ~20% and avoids any benefitless cast
    (DVE accum ops run at 1 elem/cycle regardless of dtype).
  - Single activation table set for the WHOLE kernel: one explicit
    InstLoadActFuncSet(natural_log_exp_and_others) preload; square/copy/
    ln/exp all live in that set, so zero ~2.7us table switches ever happen.
      rsqrt(x)   = exp(-0.5*ln(x))        (replaces Sqrt + DVE reciprocal)
      sigmoid(y) = 1/(1+exp(-y))          (replaces Sigmoid table)
  - Engine balance (measured: DVE STT-accum 957ns, ACT square+accum-read
    1119ns per 768-elem op): DVE takes all dots plus ~1 sumsq per chunk,
    ACT takes 13 sumsq per 14-patch chunk -> both engines total ~411us,
    just under the ~415us HBM stream.
  - g0 epilogue runs mid-stream; g1 epilogue is column-sliced so only the
    last 14 columns' short chain (~2.5us, no table loads) trails the final
    (tapered) DMA.
  - Partial sums [128, 3] DMA'd out; final reduction on host.
"""

import os
import sys

import numpy as np

if "/opt/trn_rl_repo" not in sys.path:
    sys.path.insert(0, "/opt/trn_rl_repo")

from contextlib import ExitStack

import concourse.bacc as bacc
import concourse.tile as tile
from concourse import mybir
from concourse.bass_utils import run_bass_kernel_spmd
from concourse.hw_specs import get_activation_tables

N_CORES = 8
B, P, D = 2048, 197, 768
NP = P - 1          # 196 usable patch tokens
BS = B // N_CORES   # 256 batches per core
PART = 128          # SBUF partitions
G = BS // PART      # 2 groups of 128 batches per core
GAMMA = 3.4

# chunk plans (patch counts); the tail tapers so the post-DMA tail is short.
# 14-patch chunks keep the 16 SDMA engines load-uniform (16-patch chunks
# provoked a 22% skew on engine 15).
PLAN0 = [14] * 13 + [8, 4, 2]
PLAN1 = [14] * 13 + [8, 4, 2]
SLICE1 = 182        # g1 epilogue prefix [0:182] runs during the taper

F32 = mybir.dt.float32
BF16 = mybir.dt.bfloat16
ALU = mybir.AluOpType
ACTF = mybir.ActivationFunctionType

_CACHE = {}


def _n_act(pc):
    # squares given to ACT per chunk; the rest (plus all dots) go to DVE.
    # Measured per-768-elem op: DVE STT 957ns (accum forces 1x mode; bf16
    # gives no speedup), ACT square+accum-read 1119ns. Balancing the two
    # engines puts ~34 of 392 sumsq on DVE: total ~411us each, just above
    # the ~393us DMA stream, so compute is the (thin) critical path.
    return {14: 13, 8: 6, 4: 3, 2: 1}[pc]


def _build():
    nc = bacc.Bacc("TRN2", target_bir_lowering=False, debug=False)

    pt = nc.dram_tensor("patch_tokens", [BS, P, D], F32, kind="ExternalInput")
    ot = nc.dram_tensor("out_text", [BS, D], F32, kind="ExternalInput")
    gt = nc.dram_tensor("gt", [BS, 14, 14], F32, kind="ExternalInput")
    out = nc.dram_tensor("loss_parts", [PART, G + 1], F32, kind="ExternalOutput")

    pt_ap = pt.ap()
    ot_ap = ot.ap()
    gt_ap = gt.ap().rearrange("b h w -> b (h w)")
    out_ap = out.ap()

    table_names = list(get_activation_tables(nc.m.arch).keys())
    nle_id = table_names.index("natural_log_exp_and_others")

    with ExitStack() as ctx:
        tc = ctx.enter_context(tile.TileContext(nc))
        xs = ctx.enter_context(tc.tile_pool(name="xs", bufs=4))
        persist = ctx.enter_context(tc.tile_pool(name="persist", bufs=1))
        psum = ctx.enter_context(tc.tile_pool(name="psum", bufs=1, space="PSUM"))

        trash_a = psum.tile([PART, D], F32, tag="trash_a")       # ACT main out
        trash_v = persist.tile([PART, D], F32, tag="trash_v")    # DVE main out
        trash_e = persist.tile([PART, NP], F32, tag="trash_e")   # eplg main out
        loss = persist.tile([PART, G + 1], F32, tag="loss")

        # ---- preload the single activation table set used by everything ----
        nc.scalar.add_instruction(
            mybir.InstLoadActFuncSet(
                name=nc.get_next_instruction_name(),
                act_func_set_id=nle_id,
                engine=mybir.EngineType.Activation,
            )
        )

        # ---- first pt chunk DMA leads the sync queue; prologue follows ----
        x0 = xs.tile([PART, 14, D], F32, tag="x", name="x0")
        nc.sync.dma_start(
            out=x0[:, : PLAN0[0], :], in_=pt_ap[0:PART, 1 : 1 + PLAN0[0], :]
        )

        ots, otsqs, gtts, ws = [], [], [], []
        for g in range(G):
            b0 = g * PART
            otile = persist.tile([PART, D], F32, tag=f"ot{g}")
            nc.sync.dma_start(out=otile, in_=ot_ap[b0 : b0 + PART, :])
            ots.append(otile)
            gtt = persist.tile([PART, NP], F32, tag=f"gtt{g}")
            nc.sync.dma_start(out=gtt, in_=gt_ap[b0 : b0 + PART, :])
            gtts.append(gtt)
        for g in range(G):
            otsq = persist.tile([PART, 1], F32, tag=f"otsq{g}")
            nc.scalar.activation(
                out=trash_a, in_=ots[g], func=ACTF.Square, accum_out=otsq
            )
            otsqs.append(otsq)
            w = persist.tile([PART, NP], F32, tag=f"w{g}")  # -(GAMMA*gtv + 1)
            nc.scalar.activation(
                out=w, in_=gtts[g], func=ACTF.Copy, scale=-GAMMA / 255.0, bias=-1.0
            )
            ws.append(w)

        sss = [
            persist.tile([PART, NP], F32, tag=f"ss{g}", name=f"ss{g}")
            for g in range(G)
        ]
        dts = [
            persist.tile([PART, NP], F32, tag=f"dt{g}", name=f"dt{g}")
            for g in range(G)
        ]

        def chunk(g, p0, pc, x=None):
            b0 = g * PART
            if x is None:
                x = xs.tile([PART, 14, D], F32, tag="x")
                nc.sync.dma_start(
                    out=x[:, :pc, :],
                    in_=pt_ap[b0 : b0 + PART, 1 + p0 : 1 + p0 + pc, :],
                )
            na = _n_act(pc)
            for j in range(pc):
                p = p0 + j
                if j < na:
                    nc.scalar.activation(
                        out=trash_a,
                        in_=x[:, j, :],
                        func=ACTF.Square,
                        accum_out=sss[g][:, p : p + 1],
                    )
                else:
                    nc.vector.scalar_tensor_tensor(
                        out=trash_v,
                        in0=x[:, j, :],
                        scalar=1.0,
                        in1=x[:, j, :],
                        op0=ALU.mult,
                        op1=ALU.mult,
                        accum_out=sss[g][:, p : p + 1],
                    )
                nc.vector.scalar_tensor_tensor(
                    out=trash_v,
                    in0=x[:, j, :],
                    scalar=1.0,
                    in1=ots[g],
                    op0=ALU.mult,
                    op1=ALU.mult,
                    accum_out=dts[g][:, p : p + 1],
                )

        # epilogue temps (per group; slices of g1 share disjoint columns)
        t1s = [
            persist.tile([PART, NP], F32, tag=f"t1_{g}", name=f"t1_{g}")
            for g in range(G)
        ]
        t2s = [
            persist.tile([PART, NP], F32, tag=f"t2_{g}", name=f"t2_{g}")
            for g in range(G)
        ]
        t3s = [
            persist.tile([PART, NP], F32, tag=f"t3_{g}", name=f"t3_{g}")
            for g in range(G)
        ]

        def epilogue(g, lo, hi, slot):
            sl = slice(lo, hi)
            t1, t2, t3 = t1s[g], t2s[g], t3s[g]
            # den2 = ss * ||ot||^2 ; rn = rsqrt(den2) = exp(-0.5*ln(den2))
            nc.vector.tensor_scalar_mul(out=t1[:, sl], in0=sss[g][:, sl], scalar1=otsqs[g])
            nc.scalar.activation(out=t2[:, sl], in_=t1[:, sl], func=ACTF.Ln)
            nc.scalar.activation(out=t1[:, sl], in_=t2[:, sl], func=ACTF.Exp, scale=-0.5)
            # cos = dot * rn ; e = exp(cos) ; 1+exp(cos-1) = e*exp(-1) + 1
            nc.vector.tensor_mul(out=t2[:, sl], in0=dts[g][:, sl], in1=t1[:, sl])
            nc.scalar.activation(out=t3[:, sl], in_=t2[:, sl], func=ACTF.Exp)
            nc.vector.tensor_scalar(
                out=t1[:, sl],
                in0=t3[:, sl],
                scalar1=float(np.exp(-1.0)),
                scalar2=1.0,
                op0=ALU.mult,
                op1=ALU.add,
            )
            nc.vector.reciprocal_approx_fast(out=t2[:, sl], in_=t1[:, sl])
            # omd = gtv - v  (<0);  lnu = ln(1 + omd);  loss += lnu * w
            nc.vector.scalar_tensor_tensor(
                out=t1[:, sl],
                in0=gtts[g][:, sl],
                scalar=1.0 / 255.0,
                in1=t2[:, sl],
                op0=ALU.mult,
                op1=ALU.subtract,
            )
            nc.scalar.activation(out=t3[:, sl], in_=t1[:, sl], func=ACTF.Ln, bias=1.0)
            nc.vector.scalar_tensor_tensor(
                out=trash_e[:, sl],
                in0=t3[:, sl],
                scalar=1.0,
                in1=ws[g][:, sl],
                op0=ALU.mult,
                op1=ALU.mult,
                accum_out=loss[:, slot : slot + 1],
            )

        # ---- both groups' streams; epilogues issued last so in-order
        # engines never stall mid-stream waiting on epilogue chains ----
        p0 = 0
        for ci, pc in enumerate(PLAN0):
            chunk(0, p0, pc, x=x0 if ci == 0 else None)
            p0 += pc
        p0 = 0
        for pc in PLAN1:
            chunk(1, p0, pc)
            p0 += pc
        epilogue(0, 0, NP, 0)
        epilogue(1, 0, SLICE1, 1)
        epilogue(1, SLICE1, NP, 2)

        nc.sync.dma_start(out=out_ap, in_=loss)

    nc.compile()
    return nc


def _get_nc():
    if "nc" not in _CACHE:
        _CACHE["nc"] = _build()
    return _CACHE["nc"]


def _run(in_maps, **kwargs):
    return run_bass_kernel_spmd(_get_nc(), in_maps, core_ids=list(range(N_CORES)), **kwargs)


def _make_in_maps(patch_tokens, out_text, gt):
    patch_tokens = np.ascontiguousarray(np.asarray(patch_tokens, dtype=np.float32))
    out_text = np.ascontiguousarray(np.asarray(out_text, dtype=np.float32))
    gt = np.ascontiguousarray(np.asarray(gt, dtype=np.float32))
    in_maps = []
    for c in range(N_CORES):
        sl = slice(c * BS, (c + 1) * BS)
        in_maps.append(
            {
                "patch_tokens": patch_tokens[sl],
                "out_text": out_text[sl],
                "gt": gt[sl],
            }
        )
    return in_maps


def kernel(patch_tokens, out_text, gt):
    res = _run(_make_in_maps(patch_tokens, out_text, gt))
    total = np.float64(0.0)
    for r in res.results:
        total += r["loss_parts"].astype(np.float64).sum()
    return np.float32(total / B)
